# revision 11
# baseline (speedup 1.0000x reference)
"""CredibilityAwareGraphAttention on 8 trn2 NeuronCores.

Sharding: core c handles batch b = c//4, query rows [qlo, qlo+1024) where
qlo = (c%4)*1024, for ALL 4 heads.  Each core owns its output rows fully,
so no cross-core reduction is needed.

Device kernel (per core), all in "scores-transposed" layout (keys on
partitions, queries on free axis):
  S^T[k,q] = K'_k . Q'_q  via one 66-deep contraction, where the
  credibility bias, 1/sqrt(dh) scale and (zero) qkv biases are folded in:
     Q'_q = [Q_q/8, cw_h*tau_q, cb_h],  K'_k = [K_k, tau_k, 1]
  E = exp(S^T)            (ScalarE, no max-subtraction: scores are O(5))
  EM = E * M^T            (VectorE, bf16; M = adjacency 0/1)
  EMOUT[h] <- EM          (bf16, unnormalized masked exp, to HBM)
  AV[d,q] += V'[k,d]^T EM (TensorE, V' = [V_h | ones] so row 64 = rowsums)
Host: computes Q/K/V projections (cheap), normalizes with the exact f32
rowsums, averages heads, applies Wo.
"""

import math
import os
import sys

import numpy as np

if "/opt/trn_rl_repo" not in sys.path:
    sys.path.insert(0, "/opt/trn_rl_repo")

import ml_dtypes

BF16 = ml_dtypes.bfloat16

B, N, H, HEADS, DH = 2, 4096, 256, 4, 64
SCALE = math.sqrt(DH)
NCORES = 8
QB = 1024  # query rows per core
KTILES = N // 128
DAUG = DH + 2  # augmented contraction depth

_CACHE = {}
_last_exec_ns = None
_last_profile = None


def _install_profile_hook():
    """Provide antenv.axon_hooks (absent in this image) so
    run_bass_kernel_spmd(trace=True) can capture NTFF profiles under axon.
    Returns True if the hook is available."""
    import contextlib
    import ctypes
    import types

    if "antenv.axon_hooks" in sys.modules:
        return True
    try:
        so_path = "/opt/axon/libaxon_pjrt.so"
        lib = ctypes.CDLL(so_path)
        if not hasattr(lib, "axon_start_nrt_profile"):
            return False
        lib.axon_start_nrt_profile.argtypes = [
            ctypes.POINTER(ctypes.c_int64),
            ctypes.c_size_t,
        ]
        lib.axon_start_nrt_profile.restype = ctypes.c_int64
        lib.axon_stop_nrt_profile.argtypes = [ctypes.c_char_p]
        lib.axon_stop_nrt_profile.restype = ctypes.c_int64

        @contextlib.contextmanager
        def _hook(output_dir, device_ids):
            import jax

            jax.devices()
            if device_ids:
                ids = (ctypes.c_int64 * len(device_ids))(*device_ids)
                rc = lib.axon_start_nrt_profile(ids, len(device_ids))
            else:
                rc = lib.axon_start_nrt_profile(None, 0)
            if rc != 0:
                raise RuntimeError(f"axon_start_nrt_profile rc={rc}")
            try:
                yield
            finally:
                n = lib.axon_stop_nrt_profile(str(output_dir).encode())
                if n < 0:
                    raise RuntimeError(f"axon_stop_nrt_profile rc={n}")

        state = {"hook": _hook}
        mod = types.ModuleType("antenv.axon_hooks")
        mod.get_axon_ntff_profile_hook = lambda: state["hook"]
        mod.set_axon_ntff_profile_hook = lambda h: state.__setitem__("hook", h)
        sys.modules["antenv.axon_hooks"] = mod

        from concourse import bass_utils

        bass_utils.upload_artifacts = lambda tmpdir: tmpdir
        return True
    except Exception:
        return False


def _build_nc():
    import concourse.bass as bass  # noqa: F401
    import concourse.tile as tile
    from concourse import bacc, mybir

    nc = bacc.Bacc(
        "TRN2",
        target_bir_lowering=False,
        debug=False,
        enable_asserts=True,
        num_devices=NCORES,
    )
    f32 = mybir.dt.float32
    bf16 = mybir.dt.bfloat16

    qt = nc.dram_tensor("qt", [HEADS, DAUG, QB], bf16, kind="ExternalInput").ap()
    kt = nc.dram_tensor("kt", [HEADS, DAUG, N], bf16, kind="ExternalInput").ap()
    vp = nc.dram_tensor("vp", [HEADS, KTILES, 128, DH + 1], bf16, kind="ExternalInput").ap()
    mt = nc.dram_tensor("mt", [N, QB], bf16, kind="ExternalInput").ap()
    emout = nc.dram_tensor("emout", [HEADS, N, QB], bf16, kind="ExternalOutput").ap()
    avout = nc.dram_tensor("avout", [HEADS, DH + 1, QB], f32, kind="ExternalOutput").ap()

    AV_LAG = 2  # AV matmul for tile t issues alongside S matmul for t+AV_LAG

    with tile.TileContext(nc) as tc:
        from contextlib import ExitStack

        with ExitStack() as ctx:
            singles = ctx.enter_context(tc.tile_pool(name="singles", bufs=1))
            e_pool = ctx.enter_context(tc.tile_pool(name="e", bufs=3))
            em_pool = ctx.enter_context(tc.tile_pool(name="em", bufs=AV_LAG + 4))
            av_sb_pool = ctx.enter_context(tc.tile_pool(name="avsb", bufs=2))
            ps_pool = ctx.enter_context(tc.tile_pool(name="ps", bufs=2, space="PSUM"))
            pav_pool = ctx.enter_context(tc.tile_pool(name="pav", bufs=2, space="PSUM"))

            # PE warmup: dense dependency-free matmul burst so HAM reaches
            # K=8/8 while input DMAs stream in (~3.4us of sustained PE busy).
            warm_sb = singles.tile([128, 512], bf16)
            nc.vector.memset(warm_sb, 0.0)
            warm_ps = ps_pool.tile([128, 512], mybir.dt.float32, tag="s")
            for _ in range(16):
                nc.tensor.matmul(
                    warm_ps, warm_sb[:, 0:128], warm_sb, start=True, stop=True
                )
            # Load the exp table set during the ramp, off the critical path.
            warm_act = singles.tile([1, 16], bf16)
            nc.scalar.activation(
                warm_act, warm_sb[0:1, 0:16], mybir.ActivationFunctionType.Exp
            )

            # Resident inputs as per-chunk tiles (separate tags) so each
            # consumer's dependency is its own chunk's DMA, not the whole
            # load.  Inputs go on the GpSimd SWDGE queue, in rough order of
            # first use; outputs own the Sync HWDGE queue.
            kt_sb, qt_sb, vp_sb, mt_sb = {}, {}, {}, {}

            def load_head(hd):
                kt_sb[hd] = singles.tile([DAUG, N], bf16, tag=f"kt{hd}", name=f"kt{hd}")
                nc.sync.dma_start(out=kt_sb[hd], in_=kt[hd])
                qt_sb[hd] = singles.tile([DAUG, QB], bf16, tag=f"qt{hd}", name=f"qt{hd}")
                nc.sync.dma_start(out=qt_sb[hd], in_=qt[hd])
                vp_sb[hd] = singles.tile([128, KTILES, DH + 1], bf16, tag=f"vp{hd}", name=f"vp{hd}")
                nc.sync.dma_start(
                    out=vp_sb[hd], in_=vp.rearrange("h t p d -> p h t d")[:, hd]
                )

            def load_mt(t):
                mt_sb[t] = singles.tile([128, QB], bf16, tag=f"mt{t}", name=f"mts{t}")
                nc.sync.dma_start(out=mt_sb[t], in_=mt[t * 128 : (t + 1) * 128, :])

            MT_PF = 4  # mask prefetch distance (tiles)
            load_head(0)
            for t in range(MT_PF):
                load_mt(t)

            def av_matmul(hd, t, em_t, psum_av):
                for half in range(2):
                    nc.tensor.matmul(
                        psum_av[:, half * 512 : (half + 1) * 512],
                        vp_sb[hd][:, t, :],
                        em_t[:, half * 512 : (half + 1) * 512],
                        start=(t == 0),
                        stop=(t == KTILES - 1),
                        skip_group_check=True,
                    )

            for hd in range(HEADS):
                psum_av = pav_pool.tile([DH + 1, QB], mybir.dt.float32)
                em_tiles = {}
                for t in range(KTILES):
                    psum_s = ps_pool.tile([128, QB], mybir.dt.float32, tag="s")
                    lhs = kt_sb[hd][:, t * 128 : (t + 1) * 128]
                    for half in range(2):
                        nc.tensor.matmul(
                            psum_s[:, half * 512 : (half + 1) * 512],
                            lhs,
                            qt_sb[hd][:, half * 512 : (half + 1) * 512],
                            start=True,
                            stop=True,
                        )
                    e_t = e_pool.tile([128, QB], bf16)
                    nc.scalar.activation(e_t, psum_s, mybir.ActivationFunctionType.Exp)
                    em_t = em_pool.tile([128, QB], bf16)
                    nc.vector.tensor_mul(em_t, e_t, mt_sb[t])
                    nc.sync.dma_start(
                        out=emout[hd, t * 128 : (t + 1) * 128, :], in_=em_t
                    )
                    if hd == 0 and t + MT_PF < KTILES:
                        load_mt(t + MT_PF)
                    if hd + 1 < HEADS and t == 8:
                        load_head(hd + 1)
                    em_tiles[t] = em_t
                    if t >= AV_LAG:
                        av_matmul(hd, t - AV_LAG, em_tiles.pop(t - AV_LAG), psum_av)
                for t in range(KTILES - AV_LAG, KTILES):
                    av_matmul(hd, t, em_tiles.pop(t), psum_av)
                av_sb = av_sb_pool.tile([DH + 1, QB], mybir.dt.float32)
                nc.vector.tensor_copy(av_sb, psum_av)
                nc.sync.dma_start(out=avout[hd], in_=av_sb)

    nc.compile()
    return nc


def _get_nc():
    if "nc" not in _CACHE:
        _CACHE["nc"] = _build_nc()
    return _CACHE["nc"]


def _stage_inputs(h, tau, adj_mask, Wq, bq, Wk, bk, Wv, bv, cw, cb):
    """Host-side projections + per-core shard staging."""
    in_maps = []
    per_core_meta = []
    for b in range(B):
        X = h[b]  # (N, H) f32
        Qf = X @ Wq.T + bq
        Kf = X @ Wk.T + bk
        Vf = X @ Wv.T + bv
        tb = tau[b]  # (N,)
        for r in range(4):
            qlo = r * QB
            qt = np.empty((HEADS, DAUG, QB), np.float32)
            ktm = np.empty((HEADS, DAUG, N), np.float32)
            vpm = np.empty((HEADS, N, DH + 1), np.float32)
            for hd in range(HEADS):
                qs = Qf[qlo : qlo + QB, hd * DH : (hd + 1) * DH]
                qt[hd, :DH] = qs.T / SCALE
                qt[hd, DH] = cw[hd] * tb[qlo : qlo + QB]
                qt[hd, DH + 1] = cb[hd]
                ktm[hd, :DH] = Kf[:, hd * DH : (hd + 1) * DH].T
                ktm[hd, DH] = tb
                ktm[hd, DH + 1] = 1.0
                vpm[hd, :, :DH] = Vf[:, hd * DH : (hd + 1) * DH]
                vpm[hd, :, DH] = 1.0
            mtm = adj_mask[qlo : qlo + QB, :].T.astype(np.float32)
            in_maps.append(
                {
                    "qt": qt.astype(BF16),
                    "kt": ktm.astype(BF16),
                    "vp": vpm.reshape(HEADS, KTILES, 128, DH + 1).astype(BF16),
                    "mt": mtm.astype(BF16),
                }
            )
            per_core_meta.append((b, qlo))
    return in_maps, per_core_meta


def kernel(h, tau, adj_mask, Wq, bq, Wk, bk, Wv, bv, cw, cb, Wo, bo):
    global _last_exec_ns, _last_profile
    h = np.asarray(h, np.float32)
    tau = np.asarray(tau, np.float32)
    adj_np = np.asarray(adj_mask)
    Wq = np.asarray(Wq, np.float32)
    bq = np.asarray(bq, np.float32)
    Wk = np.asarray(Wk, np.float32)
    bk = np.asarray(bk, np.float32)
    Wv = np.asarray(Wv, np.float32)
    bv = np.asarray(bv, np.float32)
    cw = np.asarray(cw, np.float32)
    cb = np.asarray(cb, np.float32)
    Wo = np.asarray(Wo, np.float32)
    bo = np.asarray(bo, np.float32)

    from concourse.bass_utils import run_bass_kernel_spmd

    nc = _get_nc()
    in_maps, meta = _stage_inputs(h, tau, adj_np, Wq, bq, Wk, bk, Wv, bv, cw, cb)

    trace = bool(int(os.environ.get("KERNEL_TRACE", "0")))
    if trace:
        trace = _install_profile_hook()
    br = run_bass_kernel_spmd(nc, in_maps, list(range(NCORES)), trace=trace)
    _last_exec_ns = br.exec_time_ns
    _last_profile = br.profile_json
    results = br.results

    h_out = np.empty((B, N, H), np.float32)
    attn_mean = np.empty((B, N, N), np.float32)
    for c, (b, qlo) in enumerate(meta):
        av = np.asarray(results[c]["avout"], np.float32)  # (HEADS, DH+1, QB)
        em = results[c]["emout"]  # (HEADS, N, QB) bf16
        rs = av[:, DH, :]  # (HEADS, QB) rowsums
        inv = 1.0 / rs
        acc = np.zeros((N, QB), np.float32)
        for hd in range(HEADS):
            acc += em[hd].astype(np.float32) * (0.25 * inv[hd])[None, :]
        attn_mean[b, qlo : qlo + QB, :] = acc.T
        hpre = (av[:, :DH, :] * inv[:, None, :]).transpose(2, 0, 1).reshape(QB, H)
        h_out[b, qlo : qlo + QB, :] = hpre @ Wo.T + bo
    return h_out, attn_mean


# revision 14
# speedup vs baseline: 1.0332x; 1.0332x over previous
"""CredibilityAwareGraphAttention on 8 trn2 NeuronCores.

Sharding: core c handles batch b = c//4, query rows [qlo, qlo+1024) where
qlo = (c%4)*1024, for ALL 4 heads.  Each core owns its output rows fully,
so no cross-core reduction is needed.

Device kernel (per core), all in "scores-transposed" layout (keys on
partitions, queries on free axis):
  S^T[k,q] = K'_k . Q'_q  via one 66-deep contraction, where the
  credibility bias, 1/sqrt(dh) scale and (zero) qkv biases are folded in:
     Q'_q = [Q_q/8, cw_h*tau_q, cb_h],  K'_k = [K_k, tau_k, 1]
  E = exp(S^T)            (ScalarE, no max-subtraction: scores are O(5))
  EM = E * M^T            (VectorE, bf16; M = adjacency 0/1)
  EMOUT[h] <- EM          (bf16, unnormalized masked exp, to HBM)
  AV[d,q] += V'[k,d]^T EM (TensorE, V' = [V_h | ones] so row 64 = rowsums)
Host: computes Q/K/V projections (cheap), normalizes with the exact f32
rowsums, averages heads, applies Wo.
"""

import math
import os
import sys

import numpy as np

if "/opt/trn_rl_repo" not in sys.path:
    sys.path.insert(0, "/opt/trn_rl_repo")

import ml_dtypes

BF16 = ml_dtypes.bfloat16

B, N, H, HEADS, DH = 2, 4096, 256, 4, 64
SCALE = math.sqrt(DH)
NCORES = 8
QB = 1024  # query rows per core
KTILES = N // 128
DAUG = DH + 2  # augmented contraction depth

_CACHE = {}
_last_exec_ns = None
_last_profile = None


def _install_profile_hook():
    """Provide antenv.axon_hooks (absent in this image) so
    run_bass_kernel_spmd(trace=True) can capture NTFF profiles under axon.
    Returns True if the hook is available."""
    import contextlib
    import ctypes
    import types

    if "antenv.axon_hooks" in sys.modules:
        return True
    try:
        so_path = "/opt/axon/libaxon_pjrt.so"
        lib = ctypes.CDLL(so_path)
        if not hasattr(lib, "axon_start_nrt_profile"):
            return False
        lib.axon_start_nrt_profile.argtypes = [
            ctypes.POINTER(ctypes.c_int64),
            ctypes.c_size_t,
        ]
        lib.axon_start_nrt_profile.restype = ctypes.c_int64
        lib.axon_stop_nrt_profile.argtypes = [ctypes.c_char_p]
        lib.axon_stop_nrt_profile.restype = ctypes.c_int64

        @contextlib.contextmanager
        def _hook(output_dir, device_ids):
            import jax

            jax.devices()
            if device_ids:
                ids = (ctypes.c_int64 * len(device_ids))(*device_ids)
                rc = lib.axon_start_nrt_profile(ids, len(device_ids))
            else:
                rc = lib.axon_start_nrt_profile(None, 0)
            if rc != 0:
                raise RuntimeError(f"axon_start_nrt_profile rc={rc}")
            try:
                yield
            finally:
                n = lib.axon_stop_nrt_profile(str(output_dir).encode())
                if n < 0:
                    raise RuntimeError(f"axon_stop_nrt_profile rc={n}")

        state = {"hook": _hook}
        mod = types.ModuleType("antenv.axon_hooks")
        mod.get_axon_ntff_profile_hook = lambda: state["hook"]
        mod.set_axon_ntff_profile_hook = lambda h: state.__setitem__("hook", h)
        sys.modules["antenv.axon_hooks"] = mod

        from concourse import bass_utils

        bass_utils.upload_artifacts = lambda tmpdir: tmpdir
        return True
    except Exception:
        return False


def _build_nc():
    import concourse.bass as bass  # noqa: F401
    import concourse.tile as tile
    from concourse import bacc, mybir

    nc = bacc.Bacc(
        "TRN2",
        target_bir_lowering=False,
        debug=False,
        enable_asserts=True,
        num_devices=NCORES,
    )
    f32 = mybir.dt.float32
    bf16 = mybir.dt.bfloat16

    qt = nc.dram_tensor("qt", [HEADS, DAUG, QB], bf16, kind="ExternalInput").ap()
    kt = nc.dram_tensor("kt", [HEADS, DAUG, N], bf16, kind="ExternalInput").ap()
    vp = nc.dram_tensor("vp", [HEADS, KTILES, 128, DH + 1], bf16, kind="ExternalInput").ap()
    mt = nc.dram_tensor("mt", [N, QB], bf16, kind="ExternalInput").ap()
    emout = nc.dram_tensor("emout", [HEADS, N, QB], bf16, kind="ExternalOutput").ap()
    avout = nc.dram_tensor("avout", [HEADS, DH + 1, QB], f32, kind="ExternalOutput").ap()

    AV_LAG = 2  # AV matmul for tile t issues alongside S matmul for t+AV_LAG

    with tile.TileContext(nc) as tc:
        from contextlib import ExitStack

        with ExitStack() as ctx:
            singles = ctx.enter_context(tc.tile_pool(name="singles", bufs=1))
            e_pool = ctx.enter_context(tc.tile_pool(name="e", bufs=3))
            em_pool = ctx.enter_context(tc.tile_pool(name="em", bufs=AV_LAG + 4))
            av_sb_pool = ctx.enter_context(tc.tile_pool(name="avsb", bufs=2))
            ps_pool = ctx.enter_context(tc.tile_pool(name="ps", bufs=2, space="PSUM"))
            pav_pool = ctx.enter_context(tc.tile_pool(name="pav", bufs=2, space="PSUM"))

            # PE warmup: dense dependency-free matmul burst so HAM reaches
            # K=8/8 while input DMAs stream in (~3.4us of sustained PE busy).
            warm_sb = singles.tile([128, 512], bf16)
            nc.vector.memset(warm_sb, 0.0)
            warm_ps = ps_pool.tile([128, 512], mybir.dt.float32, tag="s")
            for _ in range(16):
                nc.tensor.matmul(
                    warm_ps, warm_sb[:, 0:128], warm_sb, start=True, stop=True
                )
            # Load the exp table set during the ramp, off the critical path.
            warm_act = singles.tile([1, 16], bf16)
            nc.scalar.activation(
                warm_act, warm_sb[0:1, 0:16], mybir.ActivationFunctionType.Exp
            )

            # Resident inputs as per-chunk tiles (separate tags) so each
            # consumer's dependency is its own chunk's DMA, not the whole
            # load.  Inputs go on the GpSimd SWDGE queue, in rough order of
            # first use; outputs own the Sync HWDGE queue.
            kt_sb, qt_sb, vp_sb, mt_sb = {}, {}, {}, {}

            def load_head(hd):
                kt_sb[hd] = singles.tile([DAUG, N], bf16, tag=f"kt{hd}", name=f"kt{hd}")
                nc.sync.dma_start(out=kt_sb[hd], in_=kt[hd])
                qt_sb[hd] = singles.tile([DAUG, QB], bf16, tag=f"qt{hd}", name=f"qt{hd}")
                nc.sync.dma_start(out=qt_sb[hd], in_=qt[hd])
                vp_sb[hd] = singles.tile([128, KTILES, DH + 1], bf16, tag=f"vp{hd}", name=f"vp{hd}")
                nc.sync.dma_start(
                    out=vp_sb[hd], in_=vp.rearrange("h t p d -> p h t d")[:, hd]
                )

            def load_mt(t):
                mt_sb[t] = singles.tile([128, QB], bf16, tag=f"mt{t}", name=f"mts{t}")
                nc.sync.dma_start(out=mt_sb[t], in_=mt[t * 128 : (t + 1) * 128, :])

            GROUP = 8  # k-tiles per accumulation group
            NG = KTILES // GROUP
            MT_PF = 10  # mask chunks prefetched ahead (full next group + margin)

            for hd in range(HEADS):
                load_head(hd)
            for t in range(MT_PF):
                load_mt(t)

            # Per-head SBUF accumulators for [V|1]^T @ EM
            acc_sb = {
                hd: singles.tile(
                    [DH + 1, QB], mybir.dt.float32, tag=f"acc{hd}", name=f"acc{hd}"
                )
                for hd in range(HEADS)
            }

            def av_matmul(hd, t, g, em_t, psum_av):
                for half in range(2):
                    nc.tensor.matmul(
                        psum_av[:, half * 512 : (half + 1) * 512],
                        vp_sb[hd][:, t, :],
                        em_t[:, half * 512 : (half + 1) * 512],
                        start=(t == g * GROUP),
                        stop=(t == g * GROUP + GROUP - 1),
                        skip_group_check=True,
                    )

            mt_loaded = MT_PF
            for g in range(NG):
                for hd in range(HEADS):
                    psum_av = pav_pool.tile([DH + 1, QB], mybir.dt.float32)
                    em_tiles = {}
                    for t in range(g * GROUP, (g + 1) * GROUP):
                        psum_s = ps_pool.tile([128, QB], mybir.dt.float32, tag="s")
                        lhs = kt_sb[hd][:, t * 128 : (t + 1) * 128]
                        for half in range(2):
                            nc.tensor.matmul(
                                psum_s[:, half * 512 : (half + 1) * 512],
                                lhs,
                                qt_sb[hd][:, half * 512 : (half + 1) * 512],
                                start=True,
                                stop=True,
                            )
                        e_t = e_pool.tile([128, QB], bf16)
                        nc.scalar.activation(
                            e_t, psum_s, mybir.ActivationFunctionType.Exp
                        )
                        em_t = em_pool.tile([128, QB], bf16)
                        nc.vector.tensor_mul(em_t, e_t, mt_sb[t])
                        nc.sync.dma_start(
                            out=emout[hd, t * 128 : (t + 1) * 128, :], in_=em_t
                        )
                        # stream mask chunks at 1/HEADS rate (each is reused
                        # by all heads within its group)
                        if (t - g * GROUP) % 4 == 0 and mt_loaded < KTILES:
                            load_mt(mt_loaded)
                            mt_loaded += 1
                        em_tiles[t] = em_t
                        if t - g * GROUP >= AV_LAG:
                            av_matmul(hd, t - AV_LAG, g, em_tiles.pop(t - AV_LAG), psum_av)
                    for t in range((g + 1) * GROUP - AV_LAG, (g + 1) * GROUP):
                        av_matmul(hd, t, g, em_tiles.pop(t), psum_av)
                    if g == 0:
                        nc.vector.tensor_copy(acc_sb[hd], psum_av)
                    else:
                        nc.vector.tensor_add(acc_sb[hd], acc_sb[hd], psum_av)
            for hd in range(HEADS):
                nc.sync.dma_start(out=avout[hd], in_=acc_sb[hd])

    nc.compile()
    return nc


def _get_nc():
    if "nc" not in _CACHE:
        _CACHE["nc"] = _build_nc()
    return _CACHE["nc"]


def _stage_inputs(h, tau, adj_mask, Wq, bq, Wk, bk, Wv, bv, cw, cb):
    """Host-side projections + per-core shard staging."""
    in_maps = []
    per_core_meta = []
    for b in range(B):
        X = h[b]  # (N, H) f32
        Qf = X @ Wq.T + bq
        Kf = X @ Wk.T + bk
        Vf = X @ Wv.T + bv
        tb = tau[b]  # (N,)
        for r in range(4):
            qlo = r * QB
            qt = np.empty((HEADS, DAUG, QB), np.float32)
            ktm = np.empty((HEADS, DAUG, N), np.float32)
            vpm = np.empty((HEADS, N, DH + 1), np.float32)
            for hd in range(HEADS):
                qs = Qf[qlo : qlo + QB, hd * DH : (hd + 1) * DH]
                qt[hd, :DH] = qs.T / SCALE
                qt[hd, DH] = cw[hd] * tb[qlo : qlo + QB]
                qt[hd, DH + 1] = cb[hd]
                ktm[hd, :DH] = Kf[:, hd * DH : (hd + 1) * DH].T
                ktm[hd, DH] = tb
                ktm[hd, DH + 1] = 1.0
                vpm[hd, :, :DH] = Vf[:, hd * DH : (hd + 1) * DH]
                vpm[hd, :, DH] = 1.0
            mtm = adj_mask[qlo : qlo + QB, :].T.astype(np.float32)
            in_maps.append(
                {
                    "qt": qt.astype(BF16),
                    "kt": ktm.astype(BF16),
                    "vp": vpm.reshape(HEADS, KTILES, 128, DH + 1).astype(BF16),
                    "mt": mtm.astype(BF16),
                }
            )
            per_core_meta.append((b, qlo))
    return in_maps, per_core_meta


def kernel(h, tau, adj_mask, Wq, bq, Wk, bk, Wv, bv, cw, cb, Wo, bo):
    global _last_exec_ns, _last_profile
    h = np.asarray(h, np.float32)
    tau = np.asarray(tau, np.float32)
    adj_np = np.asarray(adj_mask)
    Wq = np.asarray(Wq, np.float32)
    bq = np.asarray(bq, np.float32)
    Wk = np.asarray(Wk, np.float32)
    bk = np.asarray(bk, np.float32)
    Wv = np.asarray(Wv, np.float32)
    bv = np.asarray(bv, np.float32)
    cw = np.asarray(cw, np.float32)
    cb = np.asarray(cb, np.float32)
    Wo = np.asarray(Wo, np.float32)
    bo = np.asarray(bo, np.float32)

    from concourse.bass_utils import run_bass_kernel_spmd

    nc = _get_nc()
    in_maps, meta = _stage_inputs(h, tau, adj_np, Wq, bq, Wk, bk, Wv, bv, cw, cb)

    trace = bool(int(os.environ.get("KERNEL_TRACE", "0")))
    if trace:
        trace = _install_profile_hook()
    br = run_bass_kernel_spmd(nc, in_maps, list(range(NCORES)), trace=trace)
    _last_exec_ns = br.exec_time_ns
    _last_profile = br.profile_json
    results = br.results

    h_out = np.empty((B, N, H), np.float32)
    attn_mean = np.empty((B, N, N), np.float32)
    for c, (b, qlo) in enumerate(meta):
        av = np.asarray(results[c]["avout"], np.float32)  # (HEADS, DH+1, QB)
        em = results[c]["emout"]  # (HEADS, N, QB) bf16
        rs = av[:, DH, :]  # (HEADS, QB) rowsums
        inv = 1.0 / rs
        acc = np.zeros((N, QB), np.float32)
        for hd in range(HEADS):
            acc += em[hd].astype(np.float32) * (0.25 * inv[hd])[None, :]
        attn_mean[b, qlo : qlo + QB, :] = acc.T
        hpre = (av[:, :DH, :] * inv[:, None, :]).transpose(2, 0, 1).reshape(QB, H)
        h_out[b, qlo : qlo + QB, :] = hpre @ Wo.T + bo
    return h_out, attn_mean


# revision 15
# speedup vs baseline: 1.0936x; 1.0584x over previous
"""CredibilityAwareGraphAttention on 8 trn2 NeuronCores.

Sharding: core c handles batch b = c//4, query rows [qlo, qlo+1024) where
qlo = (c%4)*1024, for ALL 4 heads.  Each core owns its output rows fully,
so no cross-core reduction is needed.

Device kernel (per core), all in "scores-transposed" layout (keys on
partitions, queries on free axis):
  S^T[k,q] = K'_k . Q'_q  via one 66-deep contraction, where the
  credibility bias, 1/sqrt(dh) scale and (zero) qkv biases are folded in:
     Q'_q = [Q_q/8, cw_h*tau_q, cb_h],  K'_k = [K_k, tau_k, 1]
  E = exp(S^T)            (ScalarE, no max-subtraction: scores are O(5))
  EM = E * M^T            (VectorE, bf16; M = adjacency 0/1)
  EMOUT[h] <- EM          (bf16, unnormalized masked exp, to HBM)
  AV[d,q] += V'[k,d]^T EM (TensorE, V' = [V_h | ones] so row 64 = rowsums)
Host: computes Q/K/V projections (cheap), normalizes with the exact f32
rowsums, averages heads, applies Wo.
"""

import math
import os
import sys

import numpy as np

if "/opt/trn_rl_repo" not in sys.path:
    sys.path.insert(0, "/opt/trn_rl_repo")

import ml_dtypes

BF16 = ml_dtypes.bfloat16

B, N, H, HEADS, DH = 2, 4096, 256, 4, 64
SCALE = math.sqrt(DH)
NCORES = 8
QB = 1024  # query rows per core
KTILES = N // 128
DAUG = DH + 2  # augmented contraction depth

_CACHE = {}
_last_exec_ns = None
_last_profile = None


def _install_profile_hook():
    """Provide antenv.axon_hooks (absent in this image) so
    run_bass_kernel_spmd(trace=True) can capture NTFF profiles under axon.
    Returns True if the hook is available."""
    import contextlib
    import ctypes
    import types

    if "antenv.axon_hooks" in sys.modules:
        return True
    try:
        so_path = "/opt/axon/libaxon_pjrt.so"
        lib = ctypes.CDLL(so_path)
        if not hasattr(lib, "axon_start_nrt_profile"):
            return False
        lib.axon_start_nrt_profile.argtypes = [
            ctypes.POINTER(ctypes.c_int64),
            ctypes.c_size_t,
        ]
        lib.axon_start_nrt_profile.restype = ctypes.c_int64
        lib.axon_stop_nrt_profile.argtypes = [ctypes.c_char_p]
        lib.axon_stop_nrt_profile.restype = ctypes.c_int64

        @contextlib.contextmanager
        def _hook(output_dir, device_ids):
            import jax

            jax.devices()
            if device_ids:
                ids = (ctypes.c_int64 * len(device_ids))(*device_ids)
                rc = lib.axon_start_nrt_profile(ids, len(device_ids))
            else:
                rc = lib.axon_start_nrt_profile(None, 0)
            if rc != 0:
                raise RuntimeError(f"axon_start_nrt_profile rc={rc}")
            try:
                yield
            finally:
                n = lib.axon_stop_nrt_profile(str(output_dir).encode())
                if n < 0:
                    raise RuntimeError(f"axon_stop_nrt_profile rc={n}")

        state = {"hook": _hook}
        mod = types.ModuleType("antenv.axon_hooks")
        mod.get_axon_ntff_profile_hook = lambda: state["hook"]
        mod.set_axon_ntff_profile_hook = lambda h: state.__setitem__("hook", h)
        sys.modules["antenv.axon_hooks"] = mod

        from concourse import bass_utils

        bass_utils.upload_artifacts = lambda tmpdir: tmpdir
        return True
    except Exception:
        return False


def _build_nc():
    import concourse.bass as bass  # noqa: F401
    import concourse.tile as tile
    from concourse import bacc, mybir

    nc = bacc.Bacc(
        "TRN2",
        target_bir_lowering=False,
        debug=False,
        enable_asserts=True,
        num_devices=NCORES,
    )
    f32 = mybir.dt.float32
    bf16 = mybir.dt.bfloat16

    qt = nc.dram_tensor("qt", [HEADS, DAUG, QB], bf16, kind="ExternalInput").ap()
    kt = nc.dram_tensor("kt", [HEADS, DAUG, N], bf16, kind="ExternalInput").ap()
    vp = nc.dram_tensor("vp", [128, HEADS, KTILES, DH + 1], bf16, kind="ExternalInput").ap()
    mt = nc.dram_tensor("mt", [N, QB], bf16, kind="ExternalInput").ap()
    emout = nc.dram_tensor("emout", [HEADS, N, QB], bf16, kind="ExternalOutput").ap()
    avout = nc.dram_tensor("avout", [HEADS, DH + 1, QB], f32, kind="ExternalOutput").ap()

    AV_LAG = 2  # AV matmul for tile t issues alongside S matmul for t+AV_LAG

    with tile.TileContext(nc) as tc:
        from contextlib import ExitStack

        with ExitStack() as ctx:
            singles = ctx.enter_context(tc.tile_pool(name="singles", bufs=1))
            e_pool = ctx.enter_context(tc.tile_pool(name="e", bufs=4))
            em_pool = ctx.enter_context(tc.tile_pool(name="em", bufs=AV_LAG + 4))
            av_sb_pool = ctx.enter_context(tc.tile_pool(name="avsb", bufs=2))
            ps_pool = ctx.enter_context(tc.tile_pool(name="ps", bufs=2, space="PSUM"))
            pav_pool = ctx.enter_context(tc.tile_pool(name="pav", bufs=2, space="PSUM"))

            # PE warmup: dense dependency-free matmul burst so HAM reaches
            # K=8/8 while input DMAs stream in (~3.4us of sustained PE busy).
            warm_sb = singles.tile([128, 512], bf16)
            nc.vector.memset(warm_sb, 0.0)
            warm_ps = ps_pool.tile([128, 512], mybir.dt.float32, tag="s")
            for _ in range(16):
                nc.tensor.matmul(
                    warm_ps, warm_sb[:, 0:128], warm_sb, start=True, stop=True
                )
            # Load the exp table set during the ramp, off the critical path.
            warm_act = singles.tile([1, 16], bf16)
            nc.scalar.activation(
                warm_act, warm_sb[0:1, 0:16], mybir.ActivationFunctionType.Exp
            )

            # Resident inputs as per-chunk tiles (separate tags) so each
            # consumer's dependency is its own chunk's DMA, not the whole
            # load.  Inputs go on the GpSimd SWDGE queue, in rough order of
            # first use; outputs own the Sync HWDGE queue.
            kt_sb, qt_sb, vp_sb, mt_sb = {}, {}, {}, {}

            def load_head(hd):
                kt_sb[hd] = singles.tile([DAUG, N], bf16, tag=f"kt{hd}", name=f"kt{hd}")
                nc.sync.dma_start(out=kt_sb[hd], in_=kt[hd])
                qt_sb[hd] = singles.tile([DAUG, QB], bf16, tag=f"qt{hd}", name=f"qt{hd}")
                nc.sync.dma_start(out=qt_sb[hd], in_=qt[hd])
                vp_sb[hd] = singles.tile([128, KTILES, DH + 1], bf16, tag=f"vp{hd}", name=f"vp{hd}")
                nc.sync.dma_start(out=vp_sb[hd], in_=vp[:, hd])

            def load_mt(t):
                mt_sb[t] = singles.tile([128, QB], bf16, tag=f"mt{t}", name=f"mts{t}")
                nc.sync.dma_start(out=mt_sb[t], in_=mt[t * 128 : (t + 1) * 128, :])

            GROUP = 8  # k-tiles per accumulation group
            NG = KTILES // GROUP
            MT_PF = 10  # mask chunks prefetched ahead (full next group + margin)

            load_head(0)
            load_mt(0)
            load_mt(1)
            load_mt(2)
            load_head(1)
            load_mt(3)
            load_mt(4)
            load_mt(5)
            load_head(2)
            load_mt(6)
            load_mt(7)
            load_mt(8)
            load_head(3)
            load_mt(9)

            # Per-head SBUF accumulators for [V|1]^T @ EM
            acc_sb = {
                hd: singles.tile(
                    [DH + 1, QB], mybir.dt.float32, tag=f"acc{hd}", name=f"acc{hd}"
                )
                for hd in range(HEADS)
            }

            def av_matmul(hd, t, g, em_t, psum_av):
                for half in range(2):
                    nc.tensor.matmul(
                        psum_av[:, half * 512 : (half + 1) * 512],
                        vp_sb[hd][:, t, :],
                        em_t[:, half * 512 : (half + 1) * 512],
                        start=(t == g * GROUP),
                        stop=(t == g * GROUP + GROUP - 1),
                        skip_group_check=True,
                    )

            mt_loaded = MT_PF
            for g in range(NG):
                for hd in range(HEADS):
                    psum_av = pav_pool.tile([DH + 1, QB], mybir.dt.float32)
                    em_tiles = {}
                    for t in range(g * GROUP, (g + 1) * GROUP):
                        psum_s = ps_pool.tile([128, QB], mybir.dt.float32, tag="s")
                        lhs = kt_sb[hd][:, t * 128 : (t + 1) * 128]
                        for half in range(2):
                            nc.tensor.matmul(
                                psum_s[:, half * 512 : (half + 1) * 512],
                                lhs,
                                qt_sb[hd][:, half * 512 : (half + 1) * 512],
                                start=True,
                                stop=True,
                            )
                        e_t = e_pool.tile([128, QB], bf16)
                        nc.scalar.activation(
                            e_t, psum_s, mybir.ActivationFunctionType.Exp
                        )
                        em_t = em_pool.tile([128, QB], bf16)
                        nc.vector.tensor_mul(em_t, e_t, mt_sb[t])
                        nc.sync.dma_start(
                            out=emout[hd, t * 128 : (t + 1) * 128, :], in_=em_t
                        )
                        # stream mask chunks at 1/HEADS rate (each is reused
                        # by all heads within its group)
                        if (t - g * GROUP) % 4 == 0 and mt_loaded < KTILES:
                            load_mt(mt_loaded)
                            mt_loaded += 1
                        em_tiles[t] = em_t
                        if t - g * GROUP >= AV_LAG:
                            av_matmul(hd, t - AV_LAG, g, em_tiles.pop(t - AV_LAG), psum_av)
                    for t in range((g + 1) * GROUP - AV_LAG, (g + 1) * GROUP):
                        av_matmul(hd, t, g, em_tiles.pop(t), psum_av)
                    if g == 0:
                        nc.vector.tensor_copy(acc_sb[hd], psum_av)
                    else:
                        nc.vector.tensor_add(acc_sb[hd], acc_sb[hd], psum_av)
            for hd in range(HEADS):
                nc.sync.dma_start(out=avout[hd], in_=acc_sb[hd])

    nc.compile()
    return nc


def _get_nc():
    if "nc" not in _CACHE:
        _CACHE["nc"] = _build_nc()
    return _CACHE["nc"]


def _stage_inputs(h, tau, adj_mask, Wq, bq, Wk, bk, Wv, bv, cw, cb):
    """Host-side projections + per-core shard staging."""
    in_maps = []
    per_core_meta = []
    for b in range(B):
        X = h[b]  # (N, H) f32
        Qf = X @ Wq.T + bq
        Kf = X @ Wk.T + bk
        Vf = X @ Wv.T + bv
        tb = tau[b]  # (N,)
        for r in range(4):
            qlo = r * QB
            qt = np.empty((HEADS, DAUG, QB), np.float32)
            ktm = np.empty((HEADS, DAUG, N), np.float32)
            vpm = np.empty((HEADS, N, DH + 1), np.float32)
            for hd in range(HEADS):
                qs = Qf[qlo : qlo + QB, hd * DH : (hd + 1) * DH]
                qt[hd, :DH] = qs.T / SCALE
                qt[hd, DH] = cw[hd] * tb[qlo : qlo + QB]
                qt[hd, DH + 1] = cb[hd]
                ktm[hd, :DH] = Kf[:, hd * DH : (hd + 1) * DH].T
                ktm[hd, DH] = tb
                ktm[hd, DH + 1] = 1.0
                vpm[hd, :, :DH] = Vf[:, hd * DH : (hd + 1) * DH]
                vpm[hd, :, DH] = 1.0
            mtm = adj_mask[qlo : qlo + QB, :].T.astype(np.float32)
            in_maps.append(
                {
                    "qt": qt.astype(BF16),
                    "kt": ktm.astype(BF16),
                    "vp": np.ascontiguousarray(vpm.reshape(HEADS, KTILES, 128, DH + 1).transpose(2, 0, 1, 3)).astype(BF16),
                    "mt": mtm.astype(BF16),
                }
            )
            per_core_meta.append((b, qlo))
    return in_maps, per_core_meta


def kernel(h, tau, adj_mask, Wq, bq, Wk, bk, Wv, bv, cw, cb, Wo, bo):
    global _last_exec_ns, _last_profile
    h = np.asarray(h, np.float32)
    tau = np.asarray(tau, np.float32)
    adj_np = np.asarray(adj_mask)
    Wq = np.asarray(Wq, np.float32)
    bq = np.asarray(bq, np.float32)
    Wk = np.asarray(Wk, np.float32)
    bk = np.asarray(bk, np.float32)
    Wv = np.asarray(Wv, np.float32)
    bv = np.asarray(bv, np.float32)
    cw = np.asarray(cw, np.float32)
    cb = np.asarray(cb, np.float32)
    Wo = np.asarray(Wo, np.float32)
    bo = np.asarray(bo, np.float32)

    from concourse.bass_utils import run_bass_kernel_spmd

    nc = _get_nc()
    in_maps, meta = _stage_inputs(h, tau, adj_np, Wq, bq, Wk, bk, Wv, bv, cw, cb)

    trace = bool(int(os.environ.get("KERNEL_TRACE", "0")))
    if trace:
        trace = _install_profile_hook()
    br = run_bass_kernel_spmd(nc, in_maps, list(range(NCORES)), trace=trace)
    _last_exec_ns = br.exec_time_ns
    _last_profile = br.profile_json
    results = br.results

    h_out = np.empty((B, N, H), np.float32)
    attn_mean = np.empty((B, N, N), np.float32)
    for c, (b, qlo) in enumerate(meta):
        av = np.asarray(results[c]["avout"], np.float32)  # (HEADS, DH+1, QB)
        em = results[c]["emout"]  # (HEADS, N, QB) bf16
        rs = av[:, DH, :]  # (HEADS, QB) rowsums
        inv = 1.0 / rs
        acc = np.zeros((N, QB), np.float32)
        for hd in range(HEADS):
            acc += em[hd].astype(np.float32) * (0.25 * inv[hd])[None, :]
        attn_mean[b, qlo : qlo + QB, :] = acc.T
        hpre = (av[:, :DH, :] * inv[:, None, :]).transpose(2, 0, 1).reshape(QB, H)
        h_out[b, qlo : qlo + QB, :] = hpre @ Wo.T + bo
    return h_out, attn_mean
